# revision 2
# baseline (speedup 1.0000x reference)
"""Grouped MLP (MoE, 8 experts, SwiGLU) — expert-parallel Bass kernel for 8 TRN2 cores.

Reference computation (per expert e, T=1024 tokens each):
    fc1  = x_e @ w1_e            # [T, 2F]
    gate, val = split(fc1)       # [T, F] each
    act  = silu(gate) * val      # [T, F]
    out  = act @ w2_e            # [T, H]

Sharding: expert-parallel — core e owns expert e entirely. No collectives.

Per-core kernel layout choices:
  * mm1 computes fc1^T directly (stationary = w1 block [h,f], moving = x^T),
    so mm2's contraction dim (f) is already on partitions — no transpose.
  * fp16 operands (fp32 PSUM accumulation): PE runs 1 cycle/row (same as
    bf16) but with ~2^-11 mantissa error; halves DMA + SBUF vs fp32r.
  * mm2 accumulates GROUP=4 f-blocks in PSUM, then a DVE add folds the
    partial into a persistent fp32 SBUF accumulator.
"""

import numpy as np
from contextlib import ExitStack

import concourse.bacc as bacc
import concourse.mybir as mybir
import concourse.tile as tile
from concourse.bass_utils import run_bass_kernel_spmd

E = 8          # experts == cores
T = 1024       # tokens per expert
H = 2048       # hidden
F = 8192       # ffn intermediate (act width)
NHB = H // 128   # 16 h-blocks (contraction blocks for mm1)
NFB = F // 128   # 64 f-blocks (act columns)
NTB = T // 128   # 8 token blocks
GROUP = 4        # f-blocks accumulated in PSUM per mm2 partial
NG = NFB // GROUP

F16 = mybir.dt.float16
F32 = mybir.dt.float32

_CACHE: dict = {}


def build_nc():
    nc = bacc.Bacc(None, target_bir_lowering=False, debug=False, num_devices=E)

    xt_d = nc.declare_dram_parameter("xt", [128, NHB, T], F16, isOutput=False)
    w1_d = nc.declare_dram_parameter("w1t", [2 * NFB, 128, NHB, 128], F16, isOutput=False)
    w2_d = nc.declare_dram_parameter("w2r", [NFB, 128, H], F16, isOutput=False)
    out_d = nc.declare_dram_parameter("out", [128, NTB, H], F32, isOutput=True)

    with ExitStack() as ctx:
        tc = ctx.enter_context(tile.TileContext(nc))
        persist = ctx.enter_context(tc.tile_pool(name="persist", bufs=1))
        w1_pool = ctx.enter_context(tc.tile_pool(name="w1", bufs=3))
        w2_pool = ctx.enter_context(tc.tile_pool(name="w2", bufs=8))
        act_pool = ctx.enter_context(tc.tile_pool(name="act", bufs=6))
        silu_pool = ctx.enter_context(tc.tile_pool(name="silu", bufs=2))
        ps1 = ctx.enter_context(tc.tile_pool(name="ps1", bufs=1, space="PSUM"))
        ps2 = ctx.enter_context(tc.tile_pool(name="ps2", bufs=2, space="PSUM"))

        xt = persist.tile([128, NHB, T], F16, tag="xt")
        nc.sync.dma_start(xt[:], xt_d[:])
        acc = persist.tile([128, NTB, H], F32, tag="acc")

        for g in range(NG):
            act_tiles = []
            w2_tiles = []
            for jj in range(GROUP):
                j = g * GROUP + jj

                w1g = w1_pool.tile([128, NHB, 128], F16, tag="w1g")
                nc.sync.dma_start(w1g[:], w1_d[j])
                w1v = w1_pool.tile([128, NHB, 128], F16, tag="w1v")
                nc.sync.dma_start(w1v[:], w1_d[NFB + j])
                w2t = w2_pool.tile([128, H], F16, tag="w2")
                nc.sync.dma_start(w2t[:], w2_d[j])
                w2_tiles.append(w2t)

                # mm1: fc1^T block [128 f, T] accumulated over 16 h-blocks.
                # Both token halves inside the h loop so consecutive matmuls
                # share the stationary weight block.
                gate_ps = ps1.tile([128, T], F32, tag="gate")
                for h in range(NHB):
                    st, sp = (h == 0), (h == NHB - 1)
                    nc.tensor.matmul(gate_ps[:, 0:512], w1g[:, h, :], xt[:, h, 0:512],
                                     start=st, stop=sp)
                    nc.tensor.matmul(gate_ps[:, 512:1024], w1g[:, h, :], xt[:, h, 512:1024],
                                     start=st, stop=sp)
                val_ps = ps1.tile([128, T], F32, tag="val")
                for h in range(NHB):
                    st, sp = (h == 0), (h == NHB - 1)
                    nc.tensor.matmul(val_ps[:, 0:512], w1v[:, h, :], xt[:, h, 0:512],
                                     start=st, stop=sp)
                    nc.tensor.matmul(val_ps[:, 512:1024], w1v[:, h, :], xt[:, h, 512:1024],
                                     start=st, stop=sp)

                # silu(gate)*val = gate*sigmoid(gate)*val, with at most one
                # PSUM operand per DVE op (Sigmoid is also CoreSim-checkable).
                sig_sb = silu_pool.tile([128, T], F16, tag="sig")
                nc.scalar.activation(sig_sb[:], gate_ps[:],
                                     mybir.ActivationFunctionType.Sigmoid)
                sv_sb = silu_pool.tile([128, T], F16, tag="sv")
                nc.vector.tensor_mul(sv_sb[:], sig_sb[:], val_ps[:])
                actt = act_pool.tile([128, T], F16, tag="actt")
                nc.vector.tensor_mul(actt[:], sv_sb[:], gate_ps[:])
                act_tiles.append(actt)

            # mm2: for each token block, accumulate this group's GROUP
            # f-blocks in PSUM ([128, 1024] H-halves, ping-pong), then fold
            # into the fp32 SBUF accumulator.
            for t in range(NTB):
                for hh in range(2):
                    outp = ps2.tile([128, 1024], F32, tag="outp")
                    for jj in range(GROUP):
                        st, sp = (jj == 0), (jj == GROUP - 1)
                        lhsT = act_tiles[jj][:, t * 128:(t + 1) * 128]
                        for q in range(2):
                            col = hh * 1024 + q * 512
                            nc.tensor.matmul(outp[:, q * 512:(q + 1) * 512], lhsT,
                                             w2_tiles[jj][:, col:col + 512],
                                             start=st, stop=sp)
                    dst = acc[:, t, hh * 1024:(hh + 1) * 1024]
                    if g == 0:
                        nc.vector.tensor_copy(dst, outp[:])
                    else:
                        nc.vector.tensor_add(dst, dst, outp[:])

        for t in range(NTB):
            nc.sync.dma_start(out_d[:, t, :], acc[:, t, :])

    nc.compile()
    return nc


def _get_nc():
    if "nc" not in _CACHE:
        _CACHE["nc"] = build_nc()
    return _CACHE["nc"]


def prep_inputs(permuted_hidden_states, w1, w2):
    """Host-side reshape/cast into the per-core DMA-friendly layouts."""
    x = np.asarray(permuted_hidden_states, dtype=np.float32)
    w1 = np.asarray(w1, dtype=np.float32)
    w2 = np.asarray(w2, dtype=np.float32)

    # xt[e][p, hb, t] = x[e*T + t, hb*128 + p]
    xt = np.ascontiguousarray(
        x.reshape(E, T, NHB, 128).transpose(0, 3, 2, 1).astype(np.float16))
    # w1t[e][jg, p, hb, fi] = w1[e, hb*128 + p, jg*128 + fi]
    w1t = np.ascontiguousarray(
        w1.reshape(E, NHB, 128, 2 * NFB, 128).transpose(0, 3, 2, 1, 4).astype(np.float16))
    # w2r[e][j, p, :] = w2[e, j*128 + p, :]
    w2r = np.ascontiguousarray(w2.reshape(E, NFB, 128, H).astype(np.float16))
    return xt, w1t, w2r


def run_cores(inputs, trace=False, **spmd_kwargs):
    xt, w1t, w2r = prep_inputs(
        inputs["permuted_hidden_states"], inputs["w1"], inputs["w2"])
    nc = _get_nc()
    in_maps = [{"xt": xt[e], "w1t": w1t[e], "w2r": w2r[e]} for e in range(E)]
    res = run_bass_kernel_spmd(nc, in_maps, list(range(E)), trace=trace, **spmd_kwargs)
    outs = [
        res.results[e]["out"].reshape(128, NTB, H).transpose(1, 0, 2).reshape(T, H)
        for e in range(E)
    ]
    full = np.concatenate(outs, axis=0).astype(np.float32)
    return full, res


def kernel(permuted_hidden_states, tokens_per_expert, w1, w2):
    full, _ = run_cores({
        "permuted_hidden_states": permuted_hidden_states,
        "w1": w1,
        "w2": w2,
    })
    return full


# revision 3
# speedup vs baseline: 1.0463x; 1.0463x over previous
"""Grouped MLP (MoE, 8 experts, SwiGLU) — expert-parallel Bass kernel for 8 TRN2 cores.

Reference computation (per expert e, T=1024 tokens each):
    fc1  = x_e @ w1_e            # [T, 2F]
    gate, val = split(fc1)       # [T, F] each
    act  = silu(gate) * val      # [T, F]
    out  = act @ w2_e            # [T, H]

Sharding: expert-parallel — core e owns expert e entirely. No collectives.

Per-core kernel layout choices:
  * mm1 computes fc1^T directly (stationary = w1 block [h,f], moving = x^T),
    so mm2's contraction dim (f) is already on partitions — no transpose.
  * fp16 operands (fp32 PSUM accumulation): PE runs 1 cycle/row (same as
    bf16) but with ~2^-11 mantissa error; halves DMA + SBUF vs fp32r.
  * mm2 accumulates GROUP=4 f-blocks in PSUM, then a DVE add folds the
    partial into a persistent fp32 SBUF accumulator.
"""

import numpy as np
from contextlib import ExitStack

import concourse.bacc as bacc
import concourse.mybir as mybir
import concourse.tile as tile
from concourse.bass_utils import run_bass_kernel_spmd

E = 8          # experts == cores
T = 1024       # tokens per expert
H = 2048       # hidden
F = 8192       # ffn intermediate (act width)
NHB = H // 128   # 16 h-blocks (contraction blocks for mm1)
NFB = F // 128   # 64 f-blocks (act columns)
NTB = T // 128   # 8 token blocks
GROUP = 4        # f-blocks accumulated in PSUM per mm2 partial
NG = NFB // GROUP

F16 = mybir.dt.float16
F32 = mybir.dt.float32

_CACHE: dict = {}


def build_nc():
    nc = bacc.Bacc(None, target_bir_lowering=False, debug=False, num_devices=E)

    xt_d = nc.declare_dram_parameter("xt", [128, NHB, T], F16, isOutput=False)
    w1_d = nc.declare_dram_parameter("w1t", [2 * NFB, 128, NHB, 128], F16, isOutput=False)
    w2_d = nc.declare_dram_parameter("w2r", [NFB, 128, H], F16, isOutput=False)
    out_d = nc.declare_dram_parameter("out", [128, NTB, H], F32, isOutput=True)

    with ExitStack() as ctx:
        tc = ctx.enter_context(tile.TileContext(nc))
        persist = ctx.enter_context(tc.tile_pool(name="persist", bufs=1))
        w1_pool = ctx.enter_context(tc.tile_pool(name="w1", bufs=3))
        w2_pool = ctx.enter_context(tc.tile_pool(name="w2", bufs=10))
        act_pool = ctx.enter_context(tc.tile_pool(name="act", bufs=10))
        silu_pool = ctx.enter_context(tc.tile_pool(name="silu", bufs=2))
        ps1 = ctx.enter_context(tc.tile_pool(name="ps1", bufs=1, space="PSUM"))
        ps2 = ctx.enter_context(tc.tile_pool(name="ps2", bufs=2, space="PSUM"))

        prefetched = {}

        def fetch_j(j):
            if j in prefetched:
                return prefetched.pop(j)
            w1g = w1_pool.tile([128, NHB, 128], F16, tag="w1g")
            nc.sync.dma_start(w1g[:], w1_d[j])
            w1v = w1_pool.tile([128, NHB, 128], F16, tag="w1v")
            nc.sync.dma_start(w1v[:], w1_d[NFB + j])
            w2t = w2_pool.tile([128, H], F16, tag="w2")
            nc.sync.dma_start(w2t[:], w2_d[j])
            return (w1g, w1v, w2t)

        # First f-block's weights before the (large) xt transfer so the PE
        # can start as early as possible.
        prefetched[0] = fetch_j(0)

        # xt as 16 per-h tiles so mm1 starts when the first h-block lands.
        xt = []
        for h in range(NHB):
            xh = persist.tile([128, T], F16, tag=f"xt{h}")
            nc.sync.dma_start(xh[:], xt_d[:, h, :])
            xt.append(xh)
        acc = persist.tile([128, NTB, H], F32, tag="acc")

        def mm1_group(g):
            act_tiles = []
            w2_tiles = []
            for jj in range(GROUP):
                j = g * GROUP + jj
                w1g, w1v, w2t = fetch_j(j)
                w2_tiles.append(w2t)

                # mm1: fc1^T block [128 f, T] accumulated over 16 h-blocks.
                # Both token halves inside the h loop so consecutive matmuls
                # share the stationary weight block.
                gate_ps = ps1.tile([128, T], F32, tag="gate")
                for h in range(NHB):
                    st, sp = (h == 0), (h == NHB - 1)
                    nc.tensor.matmul(gate_ps[:, 0:512], w1g[:, h, :], xt[h][:, 0:512],
                                     start=st, stop=sp)
                    nc.tensor.matmul(gate_ps[:, 512:1024], w1g[:, h, :], xt[h][:, 512:1024],
                                     start=st, stop=sp)
                val_ps = ps1.tile([128, T], F32, tag="val")
                for h in range(NHB):
                    st, sp = (h == 0), (h == NHB - 1)
                    nc.tensor.matmul(val_ps[:, 0:512], w1v[:, h, :], xt[h][:, 0:512],
                                     start=st, stop=sp)
                    nc.tensor.matmul(val_ps[:, 512:1024], w1v[:, h, :], xt[h][:, 512:1024],
                                     start=st, stop=sp)

                # silu(gate)*val = gate*sigmoid(gate)*val, with at most one
                # PSUM operand per DVE op (Sigmoid is also CoreSim-checkable).
                sig_sb = silu_pool.tile([128, T], F16, tag="sig")
                nc.scalar.activation(sig_sb[:], gate_ps[:],
                                     mybir.ActivationFunctionType.Sigmoid)
                sv_sb = silu_pool.tile([128, T], F16, tag="sv")
                nc.vector.tensor_mul(sv_sb[:], sig_sb[:], val_ps[:])
                actt = act_pool.tile([128, T], F16, tag="actt")
                nc.vector.tensor_mul(actt[:], sv_sb[:], gate_ps[:])
                act_tiles.append(actt)
            return act_tiles, w2_tiles

        def mm2_group(g, act_tiles, w2_tiles):
            # mm2: for each token block, accumulate this group's GROUP
            # f-blocks in PSUM ([128, 1024] H-halves, ping-pong), then fold
            # into the fp32 SBUF accumulator.
            for t in range(NTB):
                for hh in range(2):
                    outp = ps2.tile([128, 1024], F32, tag="outp")
                    for jj in range(GROUP):
                        st, sp = (jj == 0), (jj == GROUP - 1)
                        lhsT = act_tiles[jj][:, t * 128:(t + 1) * 128]
                        for q in range(2):
                            col = hh * 1024 + q * 512
                            nc.tensor.matmul(outp[:, q * 512:(q + 1) * 512], lhsT,
                                             w2_tiles[jj][:, col:col + 512],
                                             start=st, stop=sp)
                    dst = acc[:, t, hh * 1024:(hh + 1) * 1024]
                    if g == 0:
                        nc.vector.tensor_copy(dst, outp[:])
                    else:
                        nc.vector.tensor_add(dst, dst, outp[:])
                    if g == NG - 1:
                        nc.sync.dma_start(out_d[:, t, hh * 1024:(hh + 1) * 1024],
                                          dst)

        # Software pipeline: mm2(g-1) is emitted after mm1(g), giving the
        # swiglu chain of group g-1 a full mm1 group of slack before the PE
        # needs its act tiles.
        prev = None
        for g in range(NG):
            cur = mm1_group(g)
            if prev is not None:
                mm2_group(g - 1, *prev)
            prev = cur
        mm2_group(NG - 1, *prev)

    nc.compile()
    return nc


def _get_nc():
    if "nc" not in _CACHE:
        _CACHE["nc"] = build_nc()
    return _CACHE["nc"]


def prep_inputs(permuted_hidden_states, w1, w2):
    """Host-side reshape/cast into the per-core DMA-friendly layouts."""
    x = np.asarray(permuted_hidden_states, dtype=np.float32)
    w1 = np.asarray(w1, dtype=np.float32)
    w2 = np.asarray(w2, dtype=np.float32)

    # xt[e][p, hb, t] = x[e*T + t, hb*128 + p]
    xt = np.ascontiguousarray(
        x.reshape(E, T, NHB, 128).transpose(0, 3, 2, 1).astype(np.float16))
    # w1t[e][jg, p, hb, fi] = w1[e, hb*128 + p, jg*128 + fi]
    w1t = np.ascontiguousarray(
        w1.reshape(E, NHB, 128, 2 * NFB, 128).transpose(0, 3, 2, 1, 4).astype(np.float16))
    # w2r[e][j, p, :] = w2[e, j*128 + p, :]
    w2r = np.ascontiguousarray(w2.reshape(E, NFB, 128, H).astype(np.float16))
    return xt, w1t, w2r


def run_cores(inputs, trace=False, **spmd_kwargs):
    xt, w1t, w2r = prep_inputs(
        inputs["permuted_hidden_states"], inputs["w1"], inputs["w2"])
    nc = _get_nc()
    in_maps = [{"xt": xt[e], "w1t": w1t[e], "w2r": w2r[e]} for e in range(E)]
    res = run_bass_kernel_spmd(nc, in_maps, list(range(E)), trace=trace, **spmd_kwargs)
    outs = [
        res.results[e]["out"].reshape(128, NTB, H).transpose(1, 0, 2).reshape(T, H)
        for e in range(E)
    ]
    full = np.concatenate(outs, axis=0).astype(np.float32)
    return full, res


def kernel(permuted_hidden_states, tokens_per_expert, w1, w2):
    full, _ = run_cores({
        "permuted_hidden_states": permuted_hidden_states,
        "w1": w1,
        "w2": w2,
    })
    return full
